# revision 4
# baseline (speedup 1.0000x reference)
"""Single-head attention (B=4, N=4096, E=1024, H=64) on 8 TRN2 NeuronCores.

Sharding: core c = (batch b = c//2, query-half h = c%2). Each core computes the
full K/V projections for its batch and attention for its 2048 query rows.
Attention is permutation-invariant over keys, so each core receives its batch's
x pre-transposed ([E, N], embedding on partitions) with its OWN query half in
columns 0:2048 — the program is identical across cores (pure SPMD), only the
data differs.

Device pipeline per core:
  1. proj:  kT/qT = W @ x  ([64, n] layout, h on partitions), accumulated over
     8 e-chunks in PSUM; vT likewise, then PE-transposed to V-natural
     [128, 65] tiles with a fused ones-column (for softmax row sums).
  2. attention per 512-wide query block:
       S^T[nk,nq] = kT.T @ qT          (bf16 matmuls, 4 PSUM banks wide)
       P = exp(S^T / 8)                (one ScalarE pass per 4 banks -> bf16)
       O[65,512] += [V|1].T @ P        (PSUM accumulation over 32 nk-chunks;
                                        row 64 = softmax denominators)
       normalize: r = 1/O[64], PE-broadcast r across 64 partitions,
                  out = O[0:64] * r    -> DMA out as outT [64, 2048] fp32.
Host assembles out[b, half] = outT.T.
"""

import os
import sys
import tempfile

import numpy as np

import concourse.bass as bass
import concourse.tile as tile
from concourse import bacc, mybir
from concourse.bass_utils import run_bass_kernel_spmd
from concourse.masks import make_identity

B, N, E, H = 4, 4096, 1024, 64
NCORES = 8
NQ = N // 2  # query rows per core
QB = 512  # query block (free dim of attention matmuls)
NKC = N // 128  # 32 key chunks of 128
ECH = E // 128  # 8 embedding chunks of 128
NB = N // QB  # 8 projection column blocks
QBLKS = NQ // QB  # 4 query blocks per core

F32 = mybir.dt.float32
F32R = mybir.dt.float32r
BF16 = mybir.dt.bfloat16

PROJ_F32R = True  # fp32r (1.5 cyc/row) vs fp32 (2.0) for projection matmuls
SCALE = 1.0 / np.sqrt(H)


def build_kernel():
    nc = bacc.Bacc("TRN2", target_bir_lowering=False, debug=False, num_devices=NCORES)

    X_DT = F32R if PROJ_F32R else F32
    xT_d = nc.dram_tensor("xT", [E, N], X_DT, kind="ExternalInput")
    wT_d = nc.dram_tensor("wT", [E, 3 * H], X_DT, kind="ExternalInput")
    outT_d = nc.dram_tensor("outT", [H, NQ], F32, kind="ExternalOutput")

    xT = xT_d.ap().rearrange("(c p) n -> p c n", p=128)  # [128, ECH, N]
    wT = wT_d.ap().rearrange("(c p) h -> p c h", p=128)  # [128, ECH, 192]
    outT = outT_d.ap()

    with tile.TileContext(nc) as tc:
        with (
            tc.tile_pool(name="singles", bufs=1) as singles,
            tc.tile_pool(name="xpool", bufs=3) as xpool,
            tc.tile_pool(name="qkv", bufs=1) as qkv,
            tc.tile_pool(name="vstage", bufs=2) as vstage,
            tc.tile_pool(name="ppool", bufs=3) as ppool,
            tc.tile_pool(name="npool", bufs=2) as npool,
        ):
            # --- constants ---
            wT_sb = singles.tile([128, ECH, 3 * H], X_DT)
            nc.sync.dma_start(out=wT_sb[:], in_=wT)
            ident = singles.tile([H, H], BF16)
            make_identity(nc, ident[:])
            ones_h = singles.tile([1, H], F32)
            nc.vector.memset(ones_h[:], 1.0)

            # persistent activations
            kT_sb = qkv.tile([H, N], BF16)
            qT_sb = qkv.tile([H, NQ], BF16)
            v_all = qkv.tile([128, NKC, H + 1], BF16)
            nc.vector.memset(v_all[:, :, H : H + 1], 1.0)

            # --- phase 1: projections ---
            with (
                tc.tile_pool(name="proj_ps", bufs=2, space="PSUM") as proj_ps,
                tc.tile_pool(name="tr_ps", bufs=2, space="PSUM") as tr_ps,
            ):
                for nb in range(NB):
                    x_t = xpool.tile([128, ECH, QB], X_DT)
                    nc.sync.dma_start(
                        out=x_t[:], in_=xT[:, :, nb * QB : (nb + 1) * QB]
                    )
                    want_q = nb < QBLKS
                    k_ps = proj_ps.tile([H, QB], F32, tag="k_ps")
                    v_ps = proj_ps.tile([H, QB], F32, tag="v_ps")
                    q_ps = (
                        proj_ps.tile([H, QB], F32, tag="q_ps", name="q_ps")
                        if want_q
                        else None
                    )
                    for ec in range(ECH):
                        rhs = x_t[:, ec, :]
                        first, last = ec == 0, ec == ECH - 1
                        nc.tensor.matmul(
                            k_ps[:], wT_sb[:, ec, 0:H], rhs,
                            start=first, stop=last,
                        )
                        if want_q:
                            nc.tensor.matmul(
                                q_ps[:], wT_sb[:, ec, H : 2 * H], rhs,
                                start=first, stop=last,
                            )
                        nc.tensor.matmul(
                            v_ps[:], wT_sb[:, ec, 2 * H : 3 * H], rhs,
                            start=first, stop=last,
                        )
                    nsl = slice(nb * QB, (nb + 1) * QB)
                    nc.vector.tensor_copy(kT_sb[:, nsl], k_ps[:])
                    if want_q:
                        nc.vector.tensor_copy(qT_sb[:, nsl], q_ps[:])
                    # vT block -> bf16 staging -> PE transpose -> V-natural tiles
                    vT_blk = vstage.tile([H, QB], BF16)
                    nc.vector.tensor_copy(vT_blk[:], v_ps[:])
                    for j in range(QB // 128):
                        c = nb * (QB // 128) + j
                        v_tr = tr_ps.tile([128, H], BF16)
                        nc.tensor.transpose(
                            v_tr[:], vT_blk[:, j * 128 : (j + 1) * 128], ident[:]
                        )
                        nc.vector.tensor_copy(v_all[:, c, 0:H], v_tr[:])

            # --- phase 2: attention ---
            with (
                tc.tile_pool(name="s_ps", bufs=1, space="PSUM") as s_ps,
                tc.tile_pool(name="o_ps", bufs=2, space="PSUM") as o_ps,
                tc.tile_pool(name="r_ps", bufs=2, space="PSUM") as r_ps,
            ):
                GRP = 4  # nk-chunks per exp pass (PSUM banks per S tile)
                for qb in range(QBLKS):
                    qsl = slice(qb * QB, (qb + 1) * QB)
                    o_t = o_ps.tile([H + 1, QB], F32)
                    for g in range(NKC // GRP):
                        s_t = s_ps.tile([128, GRP * QB], F32)
                        for j in range(GRP):
                            c = g * GRP + j
                            nc.tensor.matmul(
                                s_t[:, j * QB : (j + 1) * QB],
                                kT_sb[:, c * 128 : (c + 1) * 128],
                                qT_sb[:, qsl],
                                start=True, stop=True,
                            )
                        p_t = ppool.tile([128, GRP * QB], BF16)
                        nc.scalar.activation(
                            p_t[:], s_t[:],
                            mybir.ActivationFunctionType.Exp,
                            scale=SCALE,
                        )
                        for j in range(GRP):
                            c = g * GRP + j
                            nc.tensor.matmul(
                                o_t[:],
                                v_all[:, c, :],
                                p_t[:, j * QB : (j + 1) * QB],
                                start=(c == 0), stop=(c == NKC - 1),
                            )
                    # normalize: r = 1/rowsum, broadcast across partitions via PE
                    r_sb = npool.tile([1, QB], F32)
                    nc.vector.reciprocal(r_sb[:], o_t[H : H + 1, :])
                    r_rep = r_ps.tile([H, QB], F32)
                    nc.tensor.matmul(r_rep[:], ones_h[:], r_sb[:], start=True, stop=True)
                    r_cp = npool.tile([H, QB], F32)
                    nc.scalar.copy(r_cp[:], r_rep[:])
                    o_n = npool.tile([H, QB], F32)
                    nc.vector.tensor_mul(o_n[:], o_t[0:H, :], r_cp[:])
                    nc.sync.dma_start(out=outT[:, qsl], in_=o_n[:])

    nc.compile()
    return nc


_NC_CACHE = {}


def _get_nc():
    if "nc" not in _NC_CACHE:
        _NC_CACHE["nc"] = build_kernel()
    return _NC_CACHE["nc"]


def _make_in_maps(x, Wk, Wq, Wv):
    wT = np.ascontiguousarray(
        np.concatenate([Wk.T, Wq.T, Wv.T], axis=1), dtype=np.float32
    )
    in_maps = []
    for c in range(NCORES):
        b, h = divmod(c, 2)
        xb = np.asarray(x[b], dtype=np.float32)
        if h == 1:
            xb = np.concatenate([xb[NQ:], xb[:NQ]], axis=0)
        in_maps.append({"xT": np.ascontiguousarray(xb.T), "wT": wT})
    return in_maps


def kernel(x, Wk, Wq, Wv, _trace=False, _tmpdir=None):
    nc = _get_nc()
    in_maps = _make_in_maps(x, Wk, Wq, Wv)
    kwargs = {}
    if _trace:
        kwargs = dict(trace=True, tmpdir=_tmpdir or tempfile.mkdtemp())
    res = run_bass_kernel_spmd(nc, in_maps, core_ids=list(range(NCORES)), **kwargs)
    out = np.empty((B, N, H), np.float32)
    for c in range(NCORES):
        b, h = divmod(c, 2)
        out[b, h * NQ : (h + 1) * NQ, :] = res.results[c]["outT"].T
    if _trace:
        return out, res
    return out


# revision 7
# speedup vs baseline: 1.5547x; 1.5547x over previous
"""Single-head attention (B=4, N=4096, E=1024, H=64) on 8 TRN2 NeuronCores.

Sharding: core c = (batch b = c//2, query-half h = c%2). Each core computes the
full K/V projections for its batch and attention for its 2048 query rows.
Attention is permutation-invariant over keys, so each core receives its batch's
x pre-transposed ([E, N], embedding on partitions) with its OWN query half in
columns 0:2048 — the program is identical across cores (pure SPMD), only the
data differs.

Device pipeline per core:
  1. proj: kT/qT/vT = W @ x ([64, n] layout), accumulated over 8 e-chunks in
     PSUM. k and v are column-packed into one PSUM bank (k -> partitions 0:64
     via tile_position (0,0), v -> 64:128 via (0,64)); q runs unpacked.
     vT drains to high partitions, is PE-transposed to V-natural [128, 65]
     tiles with a fused ones-column (for softmax row sums).
  2. attention per 512-wide query block:
       S^T[nk,nq] = kT.T @ qT          (bf16, groups of 3 PSUM banks, two
                                        groups ping-pong so ScalarE overlaps PE)
       P = exp(S^T / 8)                (one ScalarE pass per group -> bf16)
       O[65,512] += [V|1].T @ P        (PSUM accumulation over 32 nk-chunks;
                                        row 64 = softmax denominators)
       normalize: DMA row 64 to DRAM, DMA-broadcast across 64 partitions,
                  out = O[0:64] * (1/sums) -> outT [64, 2048] fp32.
Host assembles out[b, half] = outT.T.
"""

import tempfile

import numpy as np

import concourse.bass as bass
import concourse.tile as tile
from concourse import bacc, mybir
from concourse.bass_utils import run_bass_kernel_spmd
from concourse.masks import make_identity

B, N, E, H = 4, 4096, 1024, 64
NCORES = 8
NQ = N // 2  # query rows per core
QB = 512  # query block (free dim of attention matmuls)
NKC = N // 128  # 32 key chunks of 128
ECH = E // 128  # 8 embedding chunks of 128
NB = N // QB  # 8 projection column blocks
QBLKS = NQ // QB  # 4 query blocks per core

F32 = mybir.dt.float32
F32R = mybir.dt.float32r
BF16 = mybir.dt.bfloat16

SCALE = 1.0 / np.sqrt(H)


def build_kernel():
    nc = bacc.Bacc("TRN2", target_bir_lowering=False, debug=False, num_devices=NCORES)

    xT_d = nc.dram_tensor("xT", [E, N], F32, kind="ExternalInput")
    wT_d = nc.dram_tensor("wT", [E, 3 * H], F32, kind="ExternalInput")
    outT_d = nc.dram_tensor("outT", [H, NQ], F32, kind="ExternalOutput")
    sums_d = nc.dram_tensor("sums_bounce", [QBLKS, QB], F32)

    xT = xT_d.ap().rearrange("(c p) n -> p c n", p=128)  # [128, ECH, N]
    wT = wT_d.ap().rearrange("(c p) h -> p c h", p=128)  # [128, ECH, 192]
    outT = outT_d.ap()
    sums = sums_d.ap()

    with tile.TileContext(nc) as tc:
        with (
            tc.tile_pool(name="singles", bufs=1) as singles,
            tc.tile_pool(name="xpool", bufs=3) as xpool,
            tc.tile_pool(name="xbfpool", bufs=3) as xbfpool,
            tc.tile_pool(name="qkv", bufs=1) as qkv,
            tc.tile_pool(name="vstage", bufs=2) as vstage,
            tc.tile_pool(name="ppool", bufs=4) as ppool,
            tc.tile_pool(name="npool", bufs=2) as npool,
        ):
            # --- constants ---
            wT_f32 = singles.tile([128, ECH, 3 * H], F32)
            nc.sync.dma_start(out=wT_f32[:], in_=wT)
            wT_sb = singles.tile([128, ECH, 3 * H], BF16)
            nc.vector.tensor_copy(wT_sb[:], wT_f32[:])
            # identity for PE transposes, in both partition halves (v drains hi)
            ident = singles.tile([128, H], BF16)
            make_identity(nc, ident[0:H, :])
            nc.sync.dma_start(out=ident[H : 2 * H, :], in_=ident[0:H, :])

            # persistent activations
            kT_sb = qkv.tile([H, N], BF16)
            qT_sb = qkv.tile([H, NQ], BF16)
            v_all = qkv.tile([128, NKC, H + 1], BF16)
            nc.vector.memset(v_all[:, :, H : H + 1], 1.0)

            # --- phase 1: projections ---
            with (
                tc.tile_pool(name="kv_ps", bufs=2, space="PSUM") as kv_pool,
                tc.tile_pool(name="q_ps", bufs=2, space="PSUM") as q_pool,
                tc.tile_pool(name="tr_ps", bufs=2, space="PSUM") as tr_pool,
            ):
                for nb in range(NB):
                    x_t = xpool.tile([128, ECH, QB], F32)
                    nc.sync.dma_start(
                        out=x_t[:], in_=xT[:, :, nb * QB : (nb + 1) * QB]
                    )
                    x_bf = xbfpool.tile([128, ECH, QB], BF16)
                    nc.vector.tensor_copy(x_bf[:], x_t[:])
                    want_q = nb < QBLKS
                    kv_ps = kv_pool.tile([128, QB], F32)
                    q_ps = (
                        q_pool.tile([H, QB], F32, name="q_ps") if want_q else None
                    )
                    for ec in range(ECH):
                        rhs = x_bf[:, ec, :]
                        first, last = ec == 0, ec == ECH - 1
                        nc.tensor.matmul(
                            kv_ps[0:H, :], wT_sb[:, ec, 0:H], rhs,
                            start=first, stop=last, tile_position=(0, 0),
                        )
                        nc.tensor.matmul(
                            kv_ps[H:128, :], wT_sb[:, ec, 2 * H : 3 * H], rhs,
                            start=first, stop=last, tile_position=(0, H),
                        )
                        if want_q:
                            nc.tensor.matmul(
                                q_ps[:], wT_sb[:, ec, H : 2 * H], rhs,
                                start=first, stop=last,
                            )
                    nsl = slice(nb * QB, (nb + 1) * QB)
                    nc.vector.tensor_copy(kT_sb[:, nsl], kv_ps[0:H, :])
                    if want_q:
                        nc.vector.tensor_copy(qT_sb[:, nsl], q_ps[:])
                    # vT (hi partitions) -> bf16 staging -> PE transpose -> V tiles
                    vT_blk = vstage.tile([128, QB], BF16)
                    nc.vector.tensor_copy(vT_blk[H:128, :], kv_ps[H:128, :])
                    for j in range(QB // 128):
                        c = nb * (QB // 128) + j
                        v_tr = tr_pool.tile([128, H], BF16)
                        nc.tensor.transpose(
                            v_tr[:],
                            vT_blk[H:128, j * 128 : (j + 1) * 128],
                            ident[H : 2 * H, :],
                            tile_position=(H, 0),
                        )
                        nc.vector.tensor_copy(v_all[:, c, 0:H], v_tr[:])

            # --- phase 2: attention ---
            with (
                tc.tile_pool(name="s_ps", bufs=2, space="PSUM") as s_pool,
                tc.tile_pool(name="o_ps", bufs=2, space="PSUM") as o_pool,
            ):
                # 32 chunks -> groups of 3 (plus a final 2); two 3-bank tiles
                # ping-pong via bufs=2 so ScalarE overlaps the next group's MMs
                group_sizes = [3] * 10 + [2]
                for qb in range(QBLKS):
                    qsl = slice(qb * QB, (qb + 1) * QB)
                    o_t = o_pool.tile([H + 1, QB], F32)
                    c = 0
                    for gs in group_sizes:
                        s_t = s_pool.tile([128, 3 * QB], F32)
                        for j in range(gs):
                            nc.tensor.matmul(
                                s_t[:, j * QB : (j + 1) * QB],
                                kT_sb[:, (c + j) * 128 : (c + j + 1) * 128],
                                qT_sb[:, qsl],
                                start=True, stop=True,
                            )
                        p_t = ppool.tile([128, 3 * QB], BF16)
                        nc.scalar.activation(
                            p_t[:, 0 : gs * QB], s_t[:, 0 : gs * QB],
                            mybir.ActivationFunctionType.Exp,
                            scale=SCALE,
                        )
                        for j in range(gs):
                            nc.tensor.matmul(
                                o_t[:],
                                v_all[:, c + j, :],
                                p_t[:, j * QB : (j + 1) * QB],
                                start=(c + j == 0), stop=(c + j == NKC - 1),
                            )
                        c += gs
                    # normalize: broadcast row sums via DRAM, multiply by 1/sums
                    s_row = npool.tile([1, QB], F32)
                    nc.vector.tensor_copy(s_row[:], o_t[H : H + 1, :])
                    nc.sync.dma_start(out=sums[qb : qb + 1, :], in_=s_row[:])
                    s_rep = npool.tile([H, QB], F32)
                    nc.sync.dma_start(
                        out=s_rep[:],
                        in_=bass.AP(
                            tensor=sums.tensor, offset=qb * QB,
                            ap=[[0, H], [1, QB]],
                        ),
                    )
                    r_rep = npool.tile([H, QB], F32)
                    nc.vector.reciprocal(r_rep[:], s_rep[:])
                    o_n = npool.tile([H, QB], F32)
                    nc.vector.tensor_mul(o_n[:], o_t[0:H, :], r_rep[:])
                    nc.sync.dma_start(out=outT[:, qsl], in_=o_n[:])

    nc.compile()
    return nc


_NC_CACHE = {}


def _get_nc():
    if "nc" not in _NC_CACHE:
        _NC_CACHE["nc"] = build_kernel()
    return _NC_CACHE["nc"]


def _make_in_maps(x, Wk, Wq, Wv):
    wT = np.ascontiguousarray(
        np.concatenate([Wk.T, Wq.T, Wv.T], axis=1), dtype=np.float32
    )
    in_maps = []
    for c in range(NCORES):
        b, h = divmod(c, 2)
        xb = np.asarray(x[b], dtype=np.float32)
        if h == 1:
            xb = np.concatenate([xb[NQ:], xb[:NQ]], axis=0)
        in_maps.append({"xT": np.ascontiguousarray(xb.T), "wT": wT})
    return in_maps


def kernel(x, Wk, Wq, Wv, _trace=False, _tmpdir=None):
    nc = _get_nc()
    in_maps = _make_in_maps(x, Wk, Wq, Wv)
    kwargs = {}
    if _trace:
        kwargs = dict(trace=True, tmpdir=_tmpdir or tempfile.mkdtemp())
    res = run_bass_kernel_spmd(nc, in_maps, core_ids=list(range(NCORES)), **kwargs)
    out = np.empty((B, N, H), np.float32)
    for c in range(NCORES):
        b, h = divmod(c, 2)
        out[b, h * NQ : (h + 1) * NQ, :] = res.results[c]["outT"].T
    if _trace:
        return out, res
    return out


# revision 8
# speedup vs baseline: 1.7448x; 1.1223x over previous
"""Single-head attention (B=4, N=4096, E=1024, H=64) on 8 TRN2 NeuronCores.

Sharding: core c = (batch b = c//2, query-half h = c%2). Each core computes the
full K/V projections for its batch and attention for its 2048 query rows.
Attention is permutation-invariant over keys, so each core receives its batch's
x pre-transposed ([E, N], embedding on partitions) with its OWN query half in
columns 0:2048 — the program is identical across cores (pure SPMD), only the
data differs.

Device pipeline per core:
  phase A (overlapped with the DMA-bound x stream): projections
     kT/qT/vT = W @ x in [64, n] layout; k|v column-packed into one PSUM bank
     (tile_position (0,0)/(0,64)); vT drains to high partitions and is
     PE-transposed into V-natural [128, 65] tiles with a fused ones column.
     kT/qT are duplicated into partitions 64:128 (SBUF->SBUF DMA) so attention
     matmuls can run row-packed. Query block 0's attention rides along: as each
     x block's 4 key chunks complete, its S^T group + exp + PV accumulate.
  phase B: query blocks 1..3, pipelined: S^T groups of 3 PSUM banks
     (double-buffered), one exp per group on ScalarE, PV accumulation with
     [V|1] stationary (row 64 = softmax denominators).
  S^T matmuls are K=64; chunks alternate PE row-groups (tile_position (0,0) /
  (64,0)) so consecutive matmuls run concurrently in the 128x128 array.
  normalize: DMA row sums to DRAM, broadcast-DMA across 64 partitions,
  out = O[0:64] * approx(1/sums) -> outT [64, 2048] fp32.
Host assembles out[b, half] = outT.T.
"""

import tempfile

import numpy as np

import concourse.bass as bass
import concourse.tile as tile
from concourse import bacc, mybir
from concourse.bass_utils import run_bass_kernel_spmd
from concourse.masks import make_identity

B, N, E, H = 4, 4096, 1024, 64
NCORES = 8
NQ = N // 2  # query rows per core
QB = 512  # query block (free dim of attention matmuls)
NKC = N // 128  # 32 key chunks of 128
ECH = E // 128  # 8 embedding chunks of 128
NB = N // QB  # 8 projection column blocks
QBLKS = NQ // QB  # 4 query blocks per core

F32 = mybir.dt.float32
BF16 = mybir.dt.bfloat16

SCALE = 1.0 / np.sqrt(H)


def build_kernel():
    nc = bacc.Bacc("TRN2", target_bir_lowering=False, debug=False, num_devices=NCORES)

    xT_d = nc.dram_tensor("xT", [E, N], F32, kind="ExternalInput")
    wT_d = nc.dram_tensor("wT", [E, 3 * H], F32, kind="ExternalInput")
    outT_d = nc.dram_tensor("outT", [H, NQ], F32, kind="ExternalOutput")
    sums_d = nc.dram_tensor("sums_bounce", [QBLKS, QB], F32)

    xT = xT_d.ap().rearrange("(c p) n -> p c n", p=128)  # [128, ECH, N]
    wT = wT_d.ap().rearrange("(c p) h -> p c h", p=128)  # [128, ECH, 192]
    outT = outT_d.ap()
    sums = sums_d.ap()

    with tile.TileContext(nc) as tc:
        with (
            tc.tile_pool(name="singles", bufs=1) as singles,
            tc.tile_pool(name="xpool", bufs=3) as xpool,
            tc.tile_pool(name="xbfpool", bufs=3) as xbfpool,
            tc.tile_pool(name="qkv", bufs=1) as qkv,
            tc.tile_pool(name="vstage", bufs=2) as vstage,
            tc.tile_pool(name="ppool", bufs=4) as ppool,
            tc.tile_pool(name="npool", bufs=2) as npool,
        ):
            # --- constants ---
            wT_f32 = singles.tile([128, ECH, 3 * H], F32)
            nc.sync.dma_start(out=wT_f32[:], in_=wT)
            wT_sb = singles.tile([128, ECH, 3 * H], BF16)
            nc.vector.tensor_copy(wT_sb[:], wT_f32[:])
            # identity for PE transposes, in the high partition half (v drains hi)
            ident = singles.tile([128, H], BF16)
            make_identity(nc, ident[0:H, :])
            nc.sync.dma_start(out=ident[H : 2 * H, :], in_=ident[0:H, :])

            # persistent activations; rows 0:64 written by projection drains,
            # rows 64:128 are DMA duplicates enabling row-packed S^T matmuls
            kT_sb = qkv.tile([128, N], BF16)
            qT_sb = qkv.tile([128, NQ], BF16)
            v_all = qkv.tile([128, NKC, H + 1], BF16)
            nc.vector.memset(v_all[:, :, H : H + 1], 1.0)

            def s_matmul(s_slice, c, qsl):
                # chunks alternate PE row-groups -> consecutive S matmuls overlap
                lo = (c % 2) == 0
                r = slice(0, H) if lo else slice(H, 2 * H)
                nc.tensor.matmul(
                    s_slice,
                    kT_sb[r, c * 128 : (c + 1) * 128],
                    qT_sb[r, qsl],
                    start=True, stop=True,
                    tile_position=(0 if lo else H, 0),
                )

            def normalize(o_t, qb):
                s_row = npool.tile([1, QB], F32, name="s_row")
                nc.vector.tensor_copy(s_row[:], o_t[H : H + 1, :])
                nc.sync.dma_start(out=sums[qb : qb + 1, :], in_=s_row[:])
                s_rep = npool.tile([H, QB], F32, name="s_rep")
                nc.sync.dma_start(
                    out=s_rep[:],
                    in_=bass.AP(
                        tensor=sums.tensor, offset=qb * QB, ap=[[0, H], [1, QB]]
                    ),
                )
                r_rep = npool.tile([H, QB], F32, name="r_rep")
                nc.vector.reciprocal_approx_fast(out=r_rep[:], in_=s_rep[:])
                o_n = npool.tile([H, QB], F32, name="o_n")
                nc.vector.tensor_mul(o_n[:], o_t[0:H, :], r_rep[:])
                nc.sync.dma_start(
                    out=outT[:, qb * QB : (qb + 1) * QB], in_=o_n[:]
                )

            # --- phase A: projections + query block 0 ---
            qsl0 = slice(0, QB)
            with (
                tc.tile_pool(name="kv_ps", bufs=1, space="PSUM") as kv_pool,
                tc.tile_pool(name="q_ps", bufs=1, space="PSUM") as q_pool,
                tc.tile_pool(name="tr_ps", bufs=1, space="PSUM") as tr_pool,
                tc.tile_pool(name="sA_ps", bufs=1, space="PSUM") as sA_pool,
                tc.tile_pool(name="oA_ps", bufs=1, space="PSUM") as oA_pool,
            ):
                o0 = oA_pool.tile([H + 1, QB], F32)
                for nb in range(NB):
                    x_t = xpool.tile([128, ECH, QB], F32)
                    nc.sync.dma_start(
                        out=x_t[:], in_=xT[:, :, nb * QB : (nb + 1) * QB]
                    )
                    x_bf = xbfpool.tile([128, ECH, QB], BF16)
                    # alternate cast engine: DVE also drains, ACT also exps
                    if nb % 2 == 0:
                        nc.vector.tensor_copy(x_bf[:], x_t[:])
                    else:
                        nc.scalar.copy(x_bf[:], x_t[:])
                    want_q = nb < QBLKS
                    kv_ps = kv_pool.tile([128, QB], F32)
                    q_ps = (
                        q_pool.tile([H, QB], F32, name="q_ps") if want_q else None
                    )
                    for ec in range(ECH):
                        rhs = x_bf[:, ec, :]
                        first, last = ec == 0, ec == ECH - 1
                        nc.tensor.matmul(
                            kv_ps[0:H, :], wT_sb[:, ec, 0:H], rhs,
                            start=first, stop=last, tile_position=(0, 0),
                        )
                        nc.tensor.matmul(
                            kv_ps[H:128, :], wT_sb[:, ec, 2 * H : 3 * H], rhs,
                            start=first, stop=last, tile_position=(0, H),
                        )
                        if want_q:
                            nc.tensor.matmul(
                                q_ps[:], wT_sb[:, ec, H : 2 * H], rhs,
                                start=first, stop=last,
                            )
                    nsl = slice(nb * QB, (nb + 1) * QB)
                    nc.vector.tensor_copy(kT_sb[0:H, nsl], kv_ps[0:H, :])
                    nc.sync.dma_start(
                        out=kT_sb[H:128, nsl], in_=kT_sb[0:H, nsl]
                    )
                    if want_q:
                        nc.vector.tensor_copy(qT_sb[0:H, nsl], q_ps[:])
                        nc.sync.dma_start(
                            out=qT_sb[H:128, nsl], in_=qT_sb[0:H, nsl]
                        )
                    # vT (hi partitions) -> bf16 staging -> PE transpose -> V tiles
                    vT_blk = vstage.tile([128, QB], BF16)
                    nc.vector.tensor_copy(vT_blk[H:128, :], kv_ps[H:128, :])
                    for j in range(QB // 128):
                        c = nb * (QB // 128) + j
                        v_tr = tr_pool.tile([128, H], BF16)
                        nc.tensor.transpose(
                            v_tr[:],
                            vT_blk[H:128, j * 128 : (j + 1) * 128],
                            ident[H : 2 * H, :],
                            tile_position=(H, 0),
                        )
                        nc.vector.tensor_copy(v_all[:, c, 0:H], v_tr[:])
                    # query block 0 rides the projection stream: this x block's
                    # 4 key chunks -> one S group + exp + PV accumulation
                    sA = sA_pool.tile([128, 4 * QB], F32)
                    for j in range(4):
                        c = nb * 4 + j
                        s_matmul(sA[:, j * QB : (j + 1) * QB], c, qsl0)
                    p_t = ppool.tile([128, 4 * QB], BF16, name="p_t")
                    nc.scalar.activation(
                        p_t[:], sA[:], mybir.ActivationFunctionType.Exp,
                        scale=SCALE,
                    )
                    for j in range(4):
                        c = nb * 4 + j
                        nc.tensor.matmul(
                            o0[:], v_all[:, c, :], p_t[:, j * QB : (j + 1) * QB],
                            start=(c == 0), stop=(c == NKC - 1),
                        )
                normalize(o0, 0)

            # --- phase B: query blocks 1..3 ---
            with (
                tc.tile_pool(name="s_ps", bufs=2, space="PSUM") as s_pool,
                tc.tile_pool(name="o_ps", bufs=2, space="PSUM") as o_pool,
            ):
                group_sizes = [3] * 10 + [2]
                for qb in range(1, QBLKS):
                    qsl = slice(qb * QB, (qb + 1) * QB)
                    o_t = o_pool.tile([H + 1, QB], F32)
                    c = 0
                    for gs in group_sizes:
                        s_t = s_pool.tile([128, 3 * QB], F32)
                        for j in range(gs):
                            s_matmul(s_t[:, j * QB : (j + 1) * QB], c + j, qsl)
                        p_t = ppool.tile([128, 3 * QB], BF16, name="p_t")
                        nc.scalar.activation(
                            p_t[:, 0 : gs * QB], s_t[:, 0 : gs * QB],
                            mybir.ActivationFunctionType.Exp,
                            scale=SCALE,
                        )
                        for j in range(gs):
                            nc.tensor.matmul(
                                o_t[:],
                                v_all[:, c + j, :],
                                p_t[:, j * QB : (j + 1) * QB],
                                start=(c + j == 0), stop=(c + j == NKC - 1),
                            )
                        c += gs
                    normalize(o_t, qb)

    nc.compile()
    return nc


_NC_CACHE = {}


def _get_nc():
    if "nc" not in _NC_CACHE:
        _NC_CACHE["nc"] = build_kernel()
    return _NC_CACHE["nc"]


def _make_in_maps(x, Wk, Wq, Wv):
    wT = np.ascontiguousarray(
        np.concatenate([Wk.T, Wq.T, Wv.T], axis=1), dtype=np.float32
    )
    in_maps = []
    for c in range(NCORES):
        b, h = divmod(c, 2)
        xb = np.asarray(x[b], dtype=np.float32)
        if h == 1:
            xb = np.concatenate([xb[NQ:], xb[:NQ]], axis=0)
        in_maps.append({"xT": np.ascontiguousarray(xb.T), "wT": wT})
    return in_maps


def kernel(x, Wk, Wq, Wv, _trace=False, _tmpdir=None):
    nc = _get_nc()
    in_maps = _make_in_maps(x, Wk, Wq, Wv)
    kwargs = {}
    if _trace:
        kwargs = dict(trace=True, tmpdir=_tmpdir or tempfile.mkdtemp())
    res = run_bass_kernel_spmd(nc, in_maps, core_ids=list(range(NCORES)), **kwargs)
    out = np.empty((B, N, H), np.float32)
    for c in range(NCORES):
        b, h = divmod(c, 2)
        out[b, h * NQ : (h + 1) * NQ, :] = res.results[c]["outT"].T
    if _trace:
        return out, res
    return out


# revision 9
# speedup vs baseline: 1.7802x; 1.0203x over previous
"""Single-head attention (B=4, N=4096, E=1024, H=64) on 8 TRN2 NeuronCores.

Sharding: core c = (batch b = c//2, query-half h = c%2). Each core computes the
full K/V projections for its batch and attention for its 2048 query rows.
Attention is permutation-invariant over keys, so each core receives its batch's
x pre-transposed ([E, N], embedding on partitions) with its OWN query half in
columns 0:2048 — the program is identical across cores (pure SPMD), only the
data differs.

Device pipeline per core:
  phase A (overlapped with the DMA-bound x stream): projections
     kT/qT/vT = W @ x in [64, n] layout; k|v column-packed into one PSUM bank
     (tile_position (0,0)/(0,64)); vT drains to high partitions and is
     PE-transposed into V-natural [128, 65] tiles with a fused ones column.
     kT/qT are duplicated into partitions 64:128 (SBUF->SBUF DMA) so attention
     matmuls can run row-packed. Query block 0's attention rides along: as each
     x block's 4 key chunks complete, its S^T group + exp + PV accumulate.
  phase B: query blocks 1..3, pipelined: S^T groups of 3 PSUM banks
     (double-buffered), one exp per group on ScalarE, PV accumulation with
     [V|1] stationary (row 64 = softmax denominators).
  S^T matmuls are K=64; chunks alternate PE row-groups (tile_position (0,0) /
  (64,0)) so consecutive matmuls run concurrently in the 128x128 array.
  normalize: DMA row sums to DRAM, broadcast-DMA across 64 partitions,
  out = O[0:64] * approx(1/sums) -> outT [64, 2048] fp32.
Host assembles out[b, half] = outT.T.
"""

import tempfile

import numpy as np

import concourse.bass as bass
import concourse.tile as tile
from concourse import bacc, mybir
from concourse.bass_utils import run_bass_kernel_spmd
from concourse.masks import make_identity

B, N, E, H = 4, 4096, 1024, 64
NCORES = 8
NQ = N // 2  # query rows per core
QB = 512  # query block (free dim of attention matmuls)
NKC = N // 128  # 32 key chunks of 128
ECH = E // 128  # 8 embedding chunks of 128
NB = N // QB  # 8 projection column blocks
QBLKS = NQ // QB  # 4 query blocks per core

F32 = mybir.dt.float32
BF16 = mybir.dt.bfloat16

SCALE = 1.0 / np.sqrt(H)


def build_kernel():
    nc = bacc.Bacc("TRN2", target_bir_lowering=False, debug=False, num_devices=NCORES)

    xT_d = nc.dram_tensor("xT", [E, N], F32, kind="ExternalInput")
    wT_d = nc.dram_tensor("wT", [E, 3 * H], F32, kind="ExternalInput")
    outT_d = nc.dram_tensor("outT", [H, NQ], F32, kind="ExternalOutput")
    sums_d = nc.dram_tensor("sums_bounce", [QBLKS, QB], F32)

    xT = xT_d.ap().rearrange("(c p) n -> p c n", p=128)  # [128, ECH, N]
    wT = wT_d.ap().rearrange("(c p) h -> p c h", p=128)  # [128, ECH, 192]
    outT = outT_d.ap()
    sums = sums_d.ap()

    with tile.TileContext(nc) as tc:
        with (
            tc.tile_pool(name="singles", bufs=1) as singles,
            tc.tile_pool(name="xpool", bufs=3) as xpool,
            tc.tile_pool(name="xbfpool", bufs=3) as xbfpool,
            tc.tile_pool(name="qkv", bufs=1) as qkv,
            tc.tile_pool(name="vstage", bufs=2) as vstage,
            tc.tile_pool(name="ppool", bufs=5) as ppool,
            tc.tile_pool(name="npool", bufs=2) as npool,
        ):
            # --- constants (x block 0's DMA is issued first; it is the
            # critical path into the first projection matmuls) ---
            x_t0 = xpool.tile([128, ECH, QB], F32, name="x_t")
            nc.sync.dma_start(out=x_t0[:, 0:4, :], in_=xT[:, 0:4, 0:QB])
            nc.sync.dma_start(out=x_t0[:, 4:8, :], in_=xT[:, 4:8, 0:QB])
            wT_f32 = singles.tile([128, ECH, 3 * H], F32)
            nc.sync.dma_start(out=wT_f32[:], in_=wT)
            wT_sb = singles.tile([128, ECH, 3 * H], BF16)
            nc.vector.tensor_copy(wT_sb[:], wT_f32[:])
            # identity for PE transposes, in the high partition half (v drains hi)
            ident = singles.tile([128, H], BF16)
            make_identity(nc, ident[0:H, :])
            nc.sync.dma_start(out=ident[H : 2 * H, :], in_=ident[0:H, :])

            # persistent activations; rows 0:64 written by projection drains,
            # rows 64:128 are DMA duplicates enabling row-packed S^T matmuls
            kT_sb = qkv.tile([128, N], BF16)
            qT_sb = qkv.tile([128, NQ], BF16)
            v_all = qkv.tile([128, NKC, H + 1], BF16)
            nc.vector.memset(v_all[:, :, H : H + 1], 1.0)

            def s_matmul(s_slice, c, qsl):
                # chunks alternate PE row-groups -> consecutive S matmuls overlap
                lo = (c % 2) == 0
                r = slice(0, H) if lo else slice(H, 2 * H)
                nc.tensor.matmul(
                    s_slice,
                    kT_sb[r, c * 128 : (c + 1) * 128],
                    qT_sb[r, qsl],
                    start=True, stop=True,
                    tile_position=(0 if lo else H, 0),
                )

            def normalize(o_t, qb):
                s_row = npool.tile([1, QB], F32, name="s_row")
                nc.vector.tensor_copy(s_row[:], o_t[H : H + 1, :])
                nc.sync.dma_start(out=sums[qb : qb + 1, :], in_=s_row[:])
                s_rep = npool.tile([H, QB], F32, name="s_rep")
                nc.sync.dma_start(
                    out=s_rep[:],
                    in_=bass.AP(
                        tensor=sums.tensor, offset=qb * QB, ap=[[0, H], [1, QB]]
                    ),
                )
                r_rep = npool.tile([H, QB], F32, name="r_rep")
                nc.vector.reciprocal_approx_fast(out=r_rep[:], in_=s_rep[:])
                o_n = npool.tile([H, QB], F32, name="o_n")
                nc.vector.tensor_mul(o_n[:], o_t[0:H, :], r_rep[:])
                nc.sync.dma_start(
                    out=outT[:, qb * QB : (qb + 1) * QB], in_=o_n[:]
                )

            # --- phase A: projections + query block 0 ---
            qsl0 = slice(0, QB)
            with (
                tc.tile_pool(name="kv_ps", bufs=1, space="PSUM") as kv_pool,
                tc.tile_pool(name="q_ps", bufs=1, space="PSUM") as q_pool,
                tc.tile_pool(name="tr_ps", bufs=1, space="PSUM") as tr_pool,
                tc.tile_pool(name="sA_ps", bufs=1, space="PSUM") as sA_pool,
                tc.tile_pool(name="oA_ps", bufs=1, space="PSUM") as oA_pool,
            ):
                o0 = oA_pool.tile([H + 1, QB], F32)
                pv_prev = None
                for nb in range(NB):
                    if nb == 0:
                        x_t = x_t0
                    else:
                        x_t = xpool.tile([128, ECH, QB], F32, name="x_t")
                        nc.sync.dma_start(
                            out=x_t[:], in_=xT[:, :, nb * QB : (nb + 1) * QB]
                        )
                    x_bf = xbfpool.tile([128, ECH, QB], BF16)
                    # alternate cast engine: DVE also drains, ACT also exps
                    if nb % 2 == 0:
                        nc.vector.tensor_copy(x_bf[:, 0:4, :], x_t[:, 0:4, :])
                        nc.vector.tensor_copy(x_bf[:, 4:8, :], x_t[:, 4:8, :])
                    else:
                        nc.scalar.copy(x_bf[:], x_t[:])
                    want_q = nb < QBLKS
                    kv_ps = kv_pool.tile([128, QB], F32)
                    q_ps = (
                        q_pool.tile([H, QB], F32, name="q_ps") if want_q else None
                    )
                    for ec in range(ECH):
                        rhs = x_bf[:, ec, :]
                        first, last = ec == 0, ec == ECH - 1
                        nc.tensor.matmul(
                            kv_ps[0:H, :], wT_sb[:, ec, 0:H], rhs,
                            start=first, stop=last, tile_position=(0, 0),
                        )
                        nc.tensor.matmul(
                            kv_ps[H:128, :], wT_sb[:, ec, 2 * H : 3 * H], rhs,
                            start=first, stop=last, tile_position=(0, H),
                        )
                        if want_q:
                            nc.tensor.matmul(
                                q_ps[:], wT_sb[:, ec, H : 2 * H], rhs,
                                start=first, stop=last,
                            )
                    nsl = slice(nb * QB, (nb + 1) * QB)
                    nc.vector.tensor_copy(kT_sb[0:H, nsl], kv_ps[0:H, :])
                    nc.sync.dma_start(
                        out=kT_sb[H:128, nsl], in_=kT_sb[0:H, nsl]
                    )
                    if want_q:
                        nc.vector.tensor_copy(qT_sb[0:H, nsl], q_ps[:])
                        nc.sync.dma_start(
                            out=qT_sb[H:128, nsl], in_=qT_sb[0:H, nsl]
                        )
                    # vT (hi partitions) -> bf16 staging -> PE transpose -> V tiles
                    vT_blk = vstage.tile([128, QB], BF16)
                    nc.vector.tensor_copy(vT_blk[H:128, :], kv_ps[H:128, :])
                    for j in range(QB // 128):
                        c = nb * (QB // 128) + j
                        v_tr = tr_pool.tile([128, H], BF16)
                        nc.tensor.transpose(
                            v_tr[:],
                            vT_blk[H:128, j * 128 : (j + 1) * 128],
                            ident[H : 2 * H, :],
                            tile_position=(H, 0),
                        )
                        nc.vector.tensor_copy(v_all[:, c, 0:H], v_tr[:])
                    # query block 0 rides the projection stream: this x block's
                    # 4 key chunks -> one S group + exp + PV accumulation
                    sA = sA_pool.tile([128, 4 * QB], F32)
                    for j in range(4):
                        c = nb * 4 + j
                        s_matmul(sA[:, j * QB : (j + 1) * QB], c, qsl0)
                    p_t = ppool.tile([128, 4 * QB], BF16, name="p_t")
                    nc.scalar.activation(
                        p_t[:], sA[:], mybir.ActivationFunctionType.Exp,
                        scale=SCALE,
                    )
                    # PV lags one group so the PE FIFO never stalls behind exp
                    if pv_prev is not None:
                        pb, p_prev = pv_prev
                        for j in range(4):
                            c = pb * 4 + j
                            nc.tensor.matmul(
                                o0[:], v_all[:, c, :],
                                p_prev[:, j * QB : (j + 1) * QB],
                                start=(c == 0), stop=(c == NKC - 1),
                            )
                    pv_prev = (nb, p_t)
                pb, p_prev = pv_prev
                for j in range(4):
                    c = pb * 4 + j
                    nc.tensor.matmul(
                        o0[:], v_all[:, c, :], p_prev[:, j * QB : (j + 1) * QB],
                        start=(c == 0), stop=(c == NKC - 1),
                    )
                normalize(o0, 0)

            # --- phase B: query blocks 1..3 ---
            with (
                tc.tile_pool(name="s_ps", bufs=2, space="PSUM") as s_pool,
                tc.tile_pool(name="o_ps", bufs=2, space="PSUM") as o_pool,
            ):
                group_sizes = [3] * 10 + [2]
                for qb in range(1, QBLKS):
                    qsl = slice(qb * QB, (qb + 1) * QB)
                    o_t = o_pool.tile([H + 1, QB], F32)

                    def pv_group(c0, gs, p_t):
                        for j in range(gs):
                            nc.tensor.matmul(
                                o_t[:],
                                v_all[:, c0 + j, :],
                                p_t[:, j * QB : (j + 1) * QB],
                                start=(c0 + j == 0), stop=(c0 + j == NKC - 1),
                            )

                    c = 0
                    pv_lag = None
                    for gs in group_sizes:
                        s_t = s_pool.tile([128, 3 * QB], F32)
                        for j in range(gs):
                            s_matmul(s_t[:, j * QB : (j + 1) * QB], c + j, qsl)
                        p_t = ppool.tile([128, 3 * QB], BF16, name="p_t")
                        nc.scalar.activation(
                            p_t[:, 0 : gs * QB], s_t[:, 0 : gs * QB],
                            mybir.ActivationFunctionType.Exp,
                            scale=SCALE,
                        )
                        if pv_lag is not None:
                            pv_group(*pv_lag)
                        pv_lag = (c, gs, p_t)
                        c += gs
                    pv_group(*pv_lag)
                    normalize(o_t, qb)

    nc.compile()
    return nc


_NC_CACHE = {}


def _get_nc():
    if "nc" not in _NC_CACHE:
        _NC_CACHE["nc"] = build_kernel()
    return _NC_CACHE["nc"]


def _make_in_maps(x, Wk, Wq, Wv):
    wT = np.ascontiguousarray(
        np.concatenate([Wk.T, Wq.T, Wv.T], axis=1), dtype=np.float32
    )
    in_maps = []
    for c in range(NCORES):
        b, h = divmod(c, 2)
        xb = np.asarray(x[b], dtype=np.float32)
        if h == 1:
            xb = np.concatenate([xb[NQ:], xb[:NQ]], axis=0)
        in_maps.append({"xT": np.ascontiguousarray(xb.T), "wT": wT})
    return in_maps


def kernel(x, Wk, Wq, Wv, _trace=False, _tmpdir=None):
    nc = _get_nc()
    in_maps = _make_in_maps(x, Wk, Wq, Wv)
    kwargs = {}
    if _trace:
        kwargs = dict(trace=True, tmpdir=_tmpdir or tempfile.mkdtemp())
    res = run_bass_kernel_spmd(nc, in_maps, core_ids=list(range(NCORES)), **kwargs)
    out = np.empty((B, N, H), np.float32)
    for c in range(NCORES):
        b, h = divmod(c, 2)
        out[b, h * NQ : (h + 1) * NQ, :] = res.results[c]["outT"].T
    if _trace:
        return out, res
    return out
